# revision 1
# baseline (speedup 1.0000x reference)
# BinaryLinear on 8 Trainium2 NeuronCores.
#
# y = x @ sign(W)^T + bias for x [8192, 4096] f32, W [4096, 4096] f32.
#
# Sharding: data-parallel over the 8192 tokens (1024 per core), per the
# problem's sharding hint. Each core runs one [K=4096, M=1024] x [K=4096,
# N=4096] matmul: stationary operand = x^T shard in bf16, moving operand =
# sign(W)^T in fp8e4m3 (+-1 is exact in fp8, and the PE accepts mixed
# bf16-stationary x fp8-moving at full bf16 rate), f32 PSUM accumulation.
# x -> bf16 rounding is the only approximation (~1.7e-3 relative output err).
#
# Execution goes through bass2jax/PJRT (axon): one jitted shard_map over the
# 8-core mesh. The donated output backing buffer is created on-device so no
# zero-filled bytes cross the host->device link.

import numpy as np
import ml_dtypes

N_TOKENS = 8192
IN_F = 4096
OUT_F = 4096
N_CORES = 8
TOK_SHARD = N_TOKENS // N_CORES

_C = {}


OUT_DT = "float16"  # device-side output dtype (upcast to f32 on host).
# f16 keeps D2H small; rounding f32 PSUM results to f16 adds ~3e-4 relative
# error rms on top of the ~1.7e-3 from x->bf16 — negligible.


ORIENT = "x_stat"  # "x_stat": x^T is stationary, y [tok, out] out.
#                    "w_stat": sign(W)^T is stationary, y^T [out, tok] out.


def _build_nc(
    out_dt=None,
    # 256 beats 512 by ~0.8us in the cost model: halved first-tile DMA size
    # lets the first matmuls start sooner without hurting steady-state DMA.
    max_k_tile=256,
    kxm_bufs=None,
    kxn_bufs=None,
    max_tile=512,
    free_dim=512,
    repeats=1,
    orient=None,
    psum_bufs=1,
    temps_bufs=3,
    n_warm=5,
    split_out=True,
):
    import concourse.mybir as mybir
    import concourse.tile as tile
    from concourse import bacc
    from concourse.kernels.tile_matmul import (
        composable_matmul_tile_kernel,
        dma_from_dram_kxm,
        dma_from_dram_kxn,
        dma_to_dram_mxn,
        k_pool_min_bufs,
    )

    out_dt = out_dt or OUT_DT
    orient = orient or ORIENT
    nc = bacc.Bacc("TRN2", target_bir_lowering=False, debug=False)
    x_t = nc.dram_tensor(
        "x_t", [IN_F, TOK_SHARD], mybir.dt.bfloat16, kind="ExternalInput"
    ).ap()
    w_t = nc.dram_tensor(
        "w_t", [IN_F, OUT_F], mybir.dt.float8e4, kind="ExternalInput"
    ).ap()
    out_shape = [TOK_SHARD, OUT_F] if orient == "x_stat" else [OUT_F, TOK_SHARD]
    y = nc.dram_tensor(
        "y", out_shape, getattr(mybir.dt, out_dt), kind="ExternalOutput"
    ).ap()
    with tile.TileContext(nc) as tc:
        import contextlib

        with contextlib.ExitStack() as es:
            kxm_ap, kxn_ap = (x_t, w_t) if orient == "x_stat" else (w_t, x_t)
            if n_warm:
                # PE warm-up: dependency-free dummy matmuls on memset tiles
                # run while the first input DMAs are in flight, so the real
                # matmul stream starts past the HAM/pstate ramp (the PE runs
                # at half clock until ~3.4us of sustained activity).
                warm = es.enter_context(tc.tile_pool(name="warm", bufs=1))
                warm_ps = es.enter_context(
                    tc.tile_pool(name="warm_ps", bufs=1, space="PSUM")
                )
                # one bf16 tile doubles as lhsT and rhs: a single fast memset
                # (bf16 SBUF hits the DVE 4x mode) is the only dependency, so
                # the PE starts within ~200ns of kernel entry
                w_t_ = warm.tile([128, 512], mybir.dt.bfloat16)
                nc.vector.memset(w_t_[:], 1.0)
                w_out = warm_ps.tile([128, 512], mybir.dt.float32)
                for _ in range(n_warm):
                    nc.tensor.matmul(
                        w_out[:], w_t_[:, :128], w_t_[:], start=True, stop=True
                    )
            num_bufs = kxn_bufs or k_pool_min_bufs(
                kxn_ap, max_tile_size=max_k_tile
            )
            kxm_pool = es.enter_context(
                tc.tile_pool(name="kxm_pool", bufs=kxm_bufs or num_bufs)
            )
            kxn_pool = es.enter_context(
                tc.tile_pool(name="kxn_pool", bufs=num_bufs)
            )
            import concourse.bass as bass

            for _ in range(repeats):
                kxm_producer, kxm_shape = dma_from_dram_kxm(kxm_pool, kxm_ap)
                kxn_producer, kxn_shape = dma_from_dram_kxn(kxn_pool, kxn_ap)
                extra = {}
                if split_out:
                    # Evict+store per PSUM subtile: each subtile's DRAM DMA
                    # starts right after its own PSUM->SBUF copy instead of
                    # after the whole block's 4 copies — pipelines the
                    # last-block tail and spreads output DMAs.
                    y3 = y.rearrange("(po pi) f -> pi po f", pi=128)

                    def reducer(nc_, psum, sbuf, md):
                        # alternate engines so the block's 4 evictions run
                        # pairwise-parallel (GpSimd can't read PSUM)
                        if md.m_subtile_idx % 2 == 0:
                            nc_.vector.tensor_copy(out=sbuf, in_=psum)
                        else:
                            nc_.scalar.copy(out=sbuf, in_=psum)
                        n_sz = min(
                            md.n_subtile,
                            md.n_slice_size - md.n_subtile_idx * md.n_subtile,
                        )
                        nc_.sync.dma_start(
                            y3[
                                :,
                                md.m_tile_idx * md.m_subtiles + md.m_subtile_idx,
                                bass.ds(
                                    md.n_tile_idx * md.n_tile
                                    + md.n_subtile_idx * md.n_subtile,
                                    n_sz,
                                ),
                            ],
                            sbuf[:, 0, :n_sz],
                        )

                    extra["mxn_subtile_reducer"] = reducer
                    mxn_consumer = lambda nc_, tile_, md: None
                else:
                    mxn_consumer = dma_to_dram_mxn(y)
                composable_matmul_tile_kernel(
                    tc=tc,
                    kxm_shape=kxm_shape,
                    kxn_shape=kxn_shape,
                    output_type=y.dtype,
                    kxm_producer=kxm_producer,
                    kxn_producer=kxn_producer,
                    mxn_consumer=mxn_consumer,
                    MATMUL_FREE_DIM=free_dim,
                    MAX_TILE_SIZE=max_tile,
                    MAX_K_TILE_SIZE=max_k_tile,
                    temps_n_bufs=temps_bufs,
                    psum_n_bufs=psum_bufs,
                    **extra,
                )
    nc.compile()
    return nc


def _get_nc():
    if "nc" not in _C:
        _C["nc"] = _build_nc()
    return _C["nc"]


def _get_runner():
    """Compile the 8-core jitted executable once; returns (fn, zeros_fn)."""
    if "runner" in _C:
        return _C["runner"]
    import jax
    import jax.numpy as jnp
    from jax.sharding import Mesh, NamedSharding, PartitionSpec

    import inspect

    try:
        from jax.experimental.shard_map import shard_map
    except ImportError:
        from jax import shard_map
    _rep_kw = (
        {"check_rep": False}
        if "check_rep" in inspect.signature(shard_map).parameters
        else {"check_vma": False}
    )
    import concourse.mybir as mybir
    from concourse import bass2jax
    from concourse.bass2jax import _bass_exec_p, install_neuronx_cc_hook

    nc = _get_nc()
    install_neuronx_cc_hook()

    partition_name = nc.partition_id_tensor.name if nc.partition_id_tensor else None
    in_names, out_names, out_avals = [], [], []
    for alloc in nc.m.functions[0].allocations:
        if not isinstance(alloc, mybir.MemoryLocationSet):
            continue
        name = alloc.memorylocations[0].name
        if alloc.kind == "ExternalInput":
            if name != partition_name:
                in_names.append(name)
        elif alloc.kind == "ExternalOutput":
            out_names.append(name)
            out_avals.append(
                jax.core.ShapedArray(
                    tuple(alloc.tensor_shape), mybir.dt.np(alloc.dtype)
                )
            )
    assert in_names == ["x_t", "w_t"] and out_names == ["y"], (in_names, out_names)
    all_in_names = list(in_names) + list(out_names)
    if partition_name is not None:
        all_in_names.append(partition_name)

    def _body(*args):
        operands = list(args)
        if partition_name is not None:
            operands.append(bass2jax.partition_id_tensor())
        outs = _bass_exec_p.bind(
            *operands,
            out_avals=tuple(out_avals),
            in_names=tuple(all_in_names),
            out_names=tuple(out_names),
            lowering_input_output_aliases=(),
            sim_require_finite=True,
            sim_require_nnan=True,
            nc=nc,
        )
        return tuple(outs)

    devices = jax.devices()[:N_CORES]
    mesh = Mesh(np.asarray(devices), ("core",))
    sharding = NamedSharding(mesh, PartitionSpec("core"))
    in_specs = (PartitionSpec("core"),) * 3  # x_t, w_t, y-backing
    out_specs = (PartitionSpec("core"),)
    fn = jax.jit(
        shard_map(_body, mesh=mesh, in_specs=in_specs, out_specs=out_specs,
                  **_rep_kw),
        donate_argnums=(2,),
        keep_unused=True,
    )
    out_np_dt = out_avals[0].dtype
    zeros_fn = jax.jit(
        lambda: jnp.zeros((N_TOKENS, OUT_F), out_np_dt),
        out_shardings=sharding,
    )
    _C["runner"] = (fn, zeros_fn, sharding, jax)
    return _C["runner"]


def _host_prep(x, weight):
    """sign/transpose/cast/shard on the host (cheap vs the matmul)."""
    xt = np.ascontiguousarray(np.asarray(x).T).astype(ml_dtypes.bfloat16)
    # global stacked layout for shard_map: axis0 = concat of per-core shards
    xg = np.concatenate(
        [xt[:, c * TOK_SHARD : (c + 1) * TOK_SHARD] for c in range(N_CORES)],
        axis=0,
    )
    wt = np.ascontiguousarray(np.sign(np.asarray(weight)).T).astype(
        ml_dtypes.float8_e4m3
    )
    wg = np.concatenate([wt] * N_CORES, axis=0)
    return xg, wg


def _run_spmd_fallback(x, weight):
    """Conservative path through bass_utils.run_bass_kernel_spmd (same
    underlying bass2jax/PJRT execution; pays extra host->device bytes for the
    zero-filled output backing buffers)."""
    from concourse.bass_utils import run_bass_kernel_spmd

    nc = _get_nc()
    xt = np.ascontiguousarray(np.asarray(x).T).astype(ml_dtypes.bfloat16)
    wt = np.ascontiguousarray(np.sign(np.asarray(weight)).T).astype(
        ml_dtypes.float8_e4m3
    )
    in_maps = [
        {"x_t": np.ascontiguousarray(xt[:, c * TOK_SHARD : (c + 1) * TOK_SHARD]),
         "w_t": wt}
        for c in range(N_CORES)
    ]
    res = run_bass_kernel_spmd(nc, in_maps, core_ids=list(range(N_CORES)))
    return np.concatenate([r["y"] for r in res.results], axis=0)


def kernel(x, weight, bias):
    try:
        fn, zeros_fn, sharding, jax = _get_runner()
        xg, wg = _host_prep(x, weight)
        xd = jax.device_put(xg, sharding)
        wd = jax.device_put(wg, sharding)
        y_backing = zeros_fn()
        (yd,) = fn(xd, wd, y_backing)
        # global [8192, 4096], token order preserved
        y = np.asarray(yd)
    except Exception:
        y = _run_spmd_fallback(x, weight)
    # upcast + bias on host
    y = y.astype(np.float32)
    y += np.asarray(bias, dtype=np.float32)[None, :]
    return y



# revision 2
# speedup vs baseline: 1.1965x; 1.1965x over previous
# BinaryLinear on 8 Trainium2 NeuronCores.
#
# y = x @ sign(W)^T + bias for x [8192, 4096] f32, W [4096, 4096] f32.
#
# Sharding: data-parallel over the 8192 tokens (1024 per core). Each core
# runs one [K=4096, M=1024] x [K=4096, N=4096] matmul with a hybrid
# K-split to beat the bf16 PE roofline:
#   - K8 = NF8*256 contraction columns run as fp8e4m3 x fp8e4m3 matmuls in
#     DoubleRow perf mode (PE virtualizes to 128x256, ~2x bf16 FLOP rate).
#     +-1 weights are exact in fp8; only x pays quantization error there.
#   - the remaining KB = 4096-K8 columns run bf16(x)-stationary x
#     fp8(w)-moving at the normal 1 cycle/row rate (near-exact).
# Accumulation is fp32 in PSUM across both batches, so the only error is
# x-quantization: rel err ~= sqrt(K8/4096)*0.0266 (fp8) + ~1.7e-3 (bf16).
# NF8=7 measures 0.0175 on the reference data vs the 2e-2 gate.
#
# Execution goes through bass2jax/PJRT (axon): one jitted shard_map over the
# 8-core mesh. The donated output backing buffer is created on-device so no
# zero-filled bytes cross the host->device link.

import numpy as np
import ml_dtypes

N_TOKENS = 8192
IN_F = 4096
OUT_F = 4096
N_CORES = 8
TOK_SHARD = N_TOKENS // N_CORES

NF8 = 7  # number of 256-wide K chunks in fp8-DoubleRow (rest bf16)
K8 = NF8 * 256
KB = IN_F - K8

_C = {}


OUT_DT = "float16"  # device-side output dtype (upcast to f32 on host).
# f16 keeps D2H small; rounding f32 PSUM results to f16 adds ~3e-4 relative
# error rms on top of the x-quantization error — negligible.


def _build_nc(
    out_dt=None,
    nf8=None,
    # 256 keeps first-tile DMA small AND gives K_SUBTILES=2, which is what
    # lets the composable kernel emit DoubleRow ([128,2,*] slices) for the
    # fp8 batch.
    max_k_tile=256,
    max_tile=512,
    free_dim=512,
    repeats=1,
    psum_bufs=1,
    temps_bufs=3,
    n_warm=5,
    split_out=True,
):
    import concourse.mybir as mybir
    import concourse.tile as tile
    from concourse import bacc
    from concourse.kernels.tile_matmul import (
        batched_producer_kxm,
        batched_producer_kxn,
        composable_matmul_tile_kernel,
        dma_from_dram_kxm,
        dma_from_dram_kxn,
        dma_to_dram_mxn,
    )

    out_dt = out_dt or OUT_DT
    nf8 = NF8 if nf8 is None else nf8
    k8 = nf8 * 256
    kb = IN_F - k8
    nc = bacc.Bacc("TRN2", target_bir_lowering=False, debug=False)
    x_parts = []  # (ap, k-slice into w)
    if k8:
        x8_t = nc.dram_tensor(
            "x8_t", [k8, TOK_SHARD], mybir.dt.float8e4, kind="ExternalInput"
        ).ap()
        x_parts.append((x8_t, slice(0, k8)))
    if kb:
        xb_t = nc.dram_tensor(
            "xb_t", [kb, TOK_SHARD], mybir.dt.bfloat16, kind="ExternalInput"
        ).ap()
        x_parts.append((xb_t, slice(k8, IN_F)))
    w_t = nc.dram_tensor(
        "w_t", [IN_F, OUT_F], mybir.dt.float8e4, kind="ExternalInput"
    ).ap()
    y = nc.dram_tensor(
        "y", [TOK_SHARD, OUT_F], getattr(mybir.dt, out_dt), kind="ExternalOutput"
    ).ap()
    with tile.TileContext(nc) as tc:
        import contextlib

        with contextlib.ExitStack() as es:
            if n_warm:
                # PE warm-up: dependency-free dummy matmuls on memset tiles
                # run while the first input DMAs are in flight, so the real
                # matmul stream starts past the HAM/pstate ramp (the PE runs
                # at half clock until ~3.4us of sustained activity).
                warm = es.enter_context(tc.tile_pool(name="warm", bufs=1))
                warm_ps = es.enter_context(
                    tc.tile_pool(name="warm_ps", bufs=1, space="PSUM")
                )
                # one bf16 tile doubles as lhsT and rhs: a single fast memset
                # (bf16 SBUF hits the DVE 4x mode) is the only dependency, so
                # the PE starts within ~200ns of kernel entry
                w_t_ = warm.tile([128, 512], mybir.dt.bfloat16)
                nc.vector.memset(w_t_[:], 1.0)
                w_out = warm_ps.tile([128, 512], mybir.dt.float32)
                for _ in range(n_warm):
                    nc.tensor.matmul(
                        w_out[:], w_t_[:, :128], w_t_[:], start=True, stop=True
                    )
            # per-batch kxm pools sized to their k-tile counts (+1 rotation
            # slack); one shared kxn pool (same tag: same shape+dtype).
            kxm_pools = [
                es.enter_context(
                    tc.tile_pool(
                        name=f"kxm_pool{i}",
                        bufs=(ap.shape[0] // max_k_tile) + 1,
                    )
                )
                for i, (ap, _) in enumerate(x_parts)
            ]
            kxn_pool = es.enter_context(
                tc.tile_pool(name="kxn_pool", bufs=(IN_F // max_k_tile) + 1)
            )
            import concourse.bass as bass

            for _ in range(repeats):
                kxm_prods, kxm_shapes = [], []
                kxn_prods, kxn_shapes = [], []
                for (ap, ksl), pool in zip(x_parts, kxm_pools):
                    p, s = dma_from_dram_kxm(pool, ap)
                    kxm_prods.append(p)
                    kxm_shapes.append(s)
                    p, s = dma_from_dram_kxn(kxn_pool, w_t[ksl])
                    kxn_prods.append(p)
                    kxn_shapes.append(s)
                kxm_producer, kxm_shape = batched_producer_kxm(
                    kxm_prods, kxm_shapes, batch_dim="k"
                )
                kxn_producer, kxn_shape = batched_producer_kxn(
                    kxn_prods, kxn_shapes, batch_dim="k"
                )
                extra = {}
                if split_out:
                    # Evict+store per PSUM subtile: each subtile's DRAM DMA
                    # starts right after its own PSUM->SBUF copy instead of
                    # after the whole block's 4 copies — pipelines the
                    # last-block tail and spreads output DMAs.
                    y3 = y.rearrange("(po pi) f -> pi po f", pi=128)

                    def reducer(nc_, psum, sbuf, md):
                        # alternate engines so the block's 4 evictions run
                        # pairwise-parallel (GpSimd can't read PSUM)
                        if md.m_subtile_idx % 2 == 0:
                            nc_.vector.tensor_copy(out=sbuf, in_=psum)
                        else:
                            nc_.scalar.copy(out=sbuf, in_=psum)
                        n_sz = min(
                            md.n_subtile,
                            md.n_slice_size - md.n_subtile_idx * md.n_subtile,
                        )
                        nc_.sync.dma_start(
                            y3[
                                :,
                                md.m_tile_idx * md.m_subtiles + md.m_subtile_idx,
                                bass.ds(
                                    md.n_tile_idx * md.n_tile
                                    + md.n_subtile_idx * md.n_subtile,
                                    n_sz,
                                ),
                            ],
                            sbuf[:, 0, :n_sz],
                        )

                    extra["mxn_subtile_reducer"] = reducer
                    mxn_consumer = lambda nc_, tile_, md: None
                else:
                    mxn_consumer = dma_to_dram_mxn(y)
                composable_matmul_tile_kernel(
                    tc=tc,
                    kxm_shape=kxm_shape,
                    kxn_shape=kxn_shape,
                    output_type=y.dtype,
                    kxm_producer=kxm_producer,
                    kxn_producer=kxn_producer,
                    mxn_consumer=mxn_consumer,
                    MATMUL_FREE_DIM=free_dim,
                    MAX_TILE_SIZE=max_tile,
                    MAX_K_TILE_SIZE=max_k_tile,
                    temps_n_bufs=temps_bufs,
                    psum_n_bufs=psum_bufs,
                    **extra,
                )
    nc.compile()
    return nc


def _get_nc():
    if "nc" not in _C:
        _C["nc"] = _build_nc()
    return _C["nc"]


def _in_names(nc):
    import concourse.mybir as mybir

    partition_name = nc.partition_id_tensor.name if nc.partition_id_tensor else None
    names = []
    for alloc in nc.m.functions[0].allocations:
        if not isinstance(alloc, mybir.MemoryLocationSet):
            continue
        name = alloc.memorylocations[0].name
        if alloc.kind == "ExternalInput" and name != partition_name:
            names.append(name)
    return names


def _get_runner():
    """Compile the 8-core jitted executable once; returns (fn, zeros_fn)."""
    if "runner" in _C:
        return _C["runner"]
    import jax
    import jax.numpy as jnp
    from jax.sharding import Mesh, NamedSharding, PartitionSpec

    import inspect

    try:
        from jax.experimental.shard_map import shard_map
    except ImportError:
        from jax import shard_map
    _rep_kw = (
        {"check_rep": False}
        if "check_rep" in inspect.signature(shard_map).parameters
        else {"check_vma": False}
    )
    import concourse.mybir as mybir
    from concourse import bass2jax
    from concourse.bass2jax import _bass_exec_p, install_neuronx_cc_hook

    nc = _get_nc()
    install_neuronx_cc_hook()

    partition_name = nc.partition_id_tensor.name if nc.partition_id_tensor else None
    in_names, out_names, out_avals = [], [], []
    for alloc in nc.m.functions[0].allocations:
        if not isinstance(alloc, mybir.MemoryLocationSet):
            continue
        name = alloc.memorylocations[0].name
        if alloc.kind == "ExternalInput":
            if name != partition_name:
                in_names.append(name)
        elif alloc.kind == "ExternalOutput":
            out_names.append(name)
            out_avals.append(
                jax.core.ShapedArray(
                    tuple(alloc.tensor_shape), mybir.dt.np(alloc.dtype)
                )
            )
    expect = (["x8_t"] if K8 else []) + (["xb_t"] if KB else []) + ["w_t"]
    assert in_names == expect and out_names == ["y"], (in_names, out_names)
    all_in_names = list(in_names) + list(out_names)
    if partition_name is not None:
        all_in_names.append(partition_name)

    def _body(*args):
        operands = list(args)
        if partition_name is not None:
            operands.append(bass2jax.partition_id_tensor())
        outs = _bass_exec_p.bind(
            *operands,
            out_avals=tuple(out_avals),
            in_names=tuple(all_in_names),
            out_names=tuple(out_names),
            lowering_input_output_aliases=(),
            sim_require_finite=True,
            sim_require_nnan=True,
            nc=nc,
        )
        return tuple(outs)

    devices = jax.devices()[:N_CORES]
    mesh = Mesh(np.asarray(devices), ("core",))
    sharding = NamedSharding(mesh, PartitionSpec("core"))
    n_args = len(in_names) + 1  # inputs + y backing
    in_specs = (PartitionSpec("core"),) * n_args
    out_specs = (PartitionSpec("core"),)
    fn = jax.jit(
        shard_map(_body, mesh=mesh, in_specs=in_specs, out_specs=out_specs,
                  **_rep_kw),
        donate_argnums=(n_args - 1,),
        keep_unused=True,
    )
    out_np_dt = out_avals[0].dtype
    zeros_fn = jax.jit(
        lambda: jnp.zeros((N_TOKENS, OUT_F), out_np_dt),
        out_shardings=sharding,
    )
    _C["runner"] = (fn, zeros_fn, sharding, jax)
    return _C["runner"]


def _shard_cols(xt):
    """[K, 8192] -> [8*K, 1024] global stacked layout for shard_map."""
    return np.concatenate(
        [xt[:, c * TOK_SHARD : (c + 1) * TOK_SHARD] for c in range(N_CORES)],
        axis=0,
    )


def _host_prep(x, weight):
    """sign/transpose/cast/shard on the host (cheap vs the matmul).

    Returns the global (8-core stacked) arrays in kernel input order:
    x8_t (fp8 K-slice), xb_t (bf16 K-slice), w_t.
    """
    xt = np.ascontiguousarray(np.asarray(x).T)
    parts = []
    if K8:
        parts.append(
            _shard_cols(xt[:K8].astype(ml_dtypes.float8_e4m3))
        )
    if KB:
        parts.append(
            _shard_cols(xt[K8:].astype(ml_dtypes.bfloat16))
        )
    wt = np.ascontiguousarray(np.sign(np.asarray(weight)).T).astype(
        ml_dtypes.float8_e4m3
    )
    parts.append(np.concatenate([wt] * N_CORES, axis=0))
    return parts


def _run_spmd_fallback(x, weight):
    """Conservative path through bass_utils.run_bass_kernel_spmd (same
    underlying bass2jax/PJRT execution; pays extra host->device bytes for the
    zero-filled output backing buffers)."""
    from concourse.bass_utils import run_bass_kernel_spmd

    nc = _get_nc()
    xt = np.ascontiguousarray(np.asarray(x).T)
    x8 = xt[:K8].astype(ml_dtypes.float8_e4m3) if K8 else None
    xb = xt[K8:].astype(ml_dtypes.bfloat16) if KB else None
    wt = np.ascontiguousarray(np.sign(np.asarray(weight)).T).astype(
        ml_dtypes.float8_e4m3
    )
    in_maps = []
    for c in range(N_CORES):
        sl = slice(c * TOK_SHARD, (c + 1) * TOK_SHARD)
        m = {}
        if x8 is not None:
            m["x8_t"] = np.ascontiguousarray(x8[:, sl])
        if xb is not None:
            m["xb_t"] = np.ascontiguousarray(xb[:, sl])
        m["w_t"] = wt
        in_maps.append(m)
    res = run_bass_kernel_spmd(nc, in_maps, core_ids=list(range(N_CORES)))
    return np.concatenate([r["y"] for r in res.results], axis=0)


def kernel(x, weight, bias):
    try:
        fn, zeros_fn, sharding, jax = _get_runner()
        parts = _host_prep(x, weight)
        args = [jax.device_put(p, sharding) for p in parts]
        args.append(zeros_fn())
        (yd,) = fn(*args)
        # global [8192, 4096], token order preserved
        y = np.asarray(yd)
    except Exception:
        y = _run_spmd_fallback(x, weight)
    # upcast + bias on host
    y = y.astype(np.float32)
    y += np.asarray(bias, dtype=np.float32)[None, :]
    return y


# revision 6
# speedup vs baseline: 1.5078x; 1.2601x over previous
# BinaryLinear on 8 Trainium2 NeuronCores.
#
# y = x @ sign(W)^T + bias for x [8192, 4096] f32, W [4096, 4096] f32.
#
# Sharding: data-parallel over the 8192 tokens (1024 per core). Each core
# runs one [K=4096, M=1024] x [K=4096, N=4096] matmul with a hybrid
# K-split to beat the bf16 PE roofline:
#   - K8 = NF8*256 contraction columns run as fp8e4m3 x fp8e4m3 matmuls in
#     DoubleRow perf mode (PE virtualizes to 128x256, ~2x bf16 FLOP rate).
#     +-1 weights are exact in fp8; only x pays quantization error there.
#   - the remaining KB = 4096-K8 columns run bf16(x)-stationary x
#     fp8(w)-moving at the normal 1 cycle/row rate (near-exact).
# Accumulation is fp32 in PSUM across both batches, so the only error is
# x-quantization: rel err ~= sqrt(K8/4096)*0.0266 (fp8) + ~1.7e-3 (bf16).
# NF8=7 measures 0.0175 on the reference data vs the 2e-2 gate.
#
# Execution goes through bass2jax/PJRT (axon): one jitted shard_map over the
# 8-core mesh. The donated output backing buffer is created on-device so no
# zero-filled bytes cross the host->device link.

import numpy as np
import ml_dtypes

N_TOKENS = 8192
IN_F = 4096
OUT_F = 4096
N_CORES = 8
TOK_SHARD = N_TOKENS // N_CORES

NF8 = 7  # number of 256-wide K chunks in fp8-DoubleRow (rest bf16)
K8 = NF8 * 256
KB = IN_F - K8

_C = {}


OUT_DT = "float16"  # device-side output dtype (upcast to f32 on host).
# f16 keeps D2H small; rounding f32 PSUM results to f16 adds ~3e-4 relative
# error rms on top of the x-quantization error — negligible.


def _build_nc(
    out_dt=None,
    nf8=None,
    # 256 keeps first-tile DMA small AND gives K_SUBTILES=2, which is what
    # lets the composable kernel emit DoubleRow ([128,2,*] slices) for the
    # fp8 batch.
    max_k_tile=256,
    max_tile=512,
    free_dim=512,
    repeats=1,
    psum_bufs=1,
    temps_bufs=3,
    n_warm=5,
    split_out=True,
    w_cache=True,
):
    import concourse.mybir as mybir
    import concourse.tile as tile
    from concourse import bacc
    from concourse.kernels.tile_matmul import (
        batched_producer_kxm,
        batched_producer_kxn,
        composable_matmul_tile_kernel,
        dma_from_dram_kxm,
        dma_from_dram_kxn,
        dma_to_dram_mxn,
    )

    out_dt = out_dt or OUT_DT
    nf8 = NF8 if nf8 is None else nf8
    k8 = nf8 * 256
    kb = IN_F - k8
    nc = bacc.Bacc("TRN2", target_bir_lowering=False, debug=False)
    x_parts = []  # (ap, k-slice into w)
    if k8:
        x8_t = nc.dram_tensor(
            "x8_t", [k8, TOK_SHARD], mybir.dt.float8e4, kind="ExternalInput"
        ).ap()
        x_parts.append((x8_t, slice(0, k8)))
    if kb:
        xb_t = nc.dram_tensor(
            "xb_t", [kb, TOK_SHARD], mybir.dt.bfloat16, kind="ExternalInput"
        ).ap()
        x_parts.append((xb_t, slice(k8, IN_F)))
    w_t = nc.dram_tensor(
        "w_t", [IN_F, OUT_F], mybir.dt.float8e4, kind="ExternalInput"
    ).ap()
    y = nc.dram_tensor(
        "y", [TOK_SHARD, OUT_F], getattr(mybir.dt, out_dt), kind="ExternalOutput"
    ).ap()
    with tile.TileContext(nc) as tc:
        import contextlib

        with contextlib.ExitStack() as es:
            if n_warm:
                # PE warm-up: dependency-free dummy matmuls on memset tiles
                # run while the first input DMAs are in flight, so the real
                # matmul stream starts past the HAM/pstate ramp (the PE runs
                # at half clock until ~3.4us of sustained activity).
                warm = es.enter_context(tc.tile_pool(name="warm", bufs=1))
                warm_ps = es.enter_context(
                    tc.tile_pool(name="warm_ps", bufs=1, space="PSUM")
                )
                # one bf16 tile doubles as lhsT and rhs: a single fast memset
                # (bf16 SBUF hits the DVE 4x mode) is the only dependency, so
                # the PE starts within ~200ns of kernel entry
                w_t_ = warm.tile([128, 512], mybir.dt.bfloat16)
                nc.vector.memset(w_t_[:], 1.0)
                w_out = warm_ps.tile([128, 512], mybir.dt.float32)
                for _ in range(n_warm):
                    nc.tensor.matmul(
                        w_out[:], w_t_[:, :128], w_t_[:], start=True, stop=True
                    )
            # per-batch kxm pools sized to their k-tile counts (+1 rotation
            # slack); one shared kxn pool (same tag: same shape+dtype).
            kxm_pools = [
                es.enter_context(
                    tc.tile_pool(
                        name=f"kxm_pool{i}",
                        bufs=(ap.shape[0] // max_k_tile) + 1,
                    )
                )
                for i, (ap, _) in enumerate(x_parts)
            ]
            kxn_pool = es.enter_context(
                tc.tile_pool(
                    name="kxn_pool",
                    bufs=1 if w_cache else (IN_F // max_k_tile) + 1,
                )
            )
            import concourse.bass as bass

            # [p, ko, n] view of w_t: row k = ko*128 + p
            w3 = w_t.rearrange("(ko ki) n -> ki ko n", ki=128)

            def make_cached_kxn(cache, loaded, base_subtile, k_dim):
                """Producer DMA-ing W k-slices into a persistent SBUF cache
                exactly once; repeat calls (second m_outer pass) return the
                cached slice with no DMA. Halves W HBM traffic vs streaming."""

                def producer(nc_, md):
                    ks = base_subtile + md.k_tile_idx * md.k_subtiles
                    dst = cache[
                        :,
                        ks : ks + md.k_subtiles,
                        md.n_tile_idx * md.n_tile : md.n_tile_idx * md.n_tile
                        + md.n_tile,
                    ]
                    key = (ks, md.n_tile_idx)
                    if key not in loaded:
                        loaded.add(key)
                        nc_.sync.dma_start(
                            dst,
                            w3[
                                :,
                                ks : ks + md.k_subtiles,
                                md.n_tile_idx * md.n_tile : md.n_tile_idx
                                * md.n_tile
                                + md.n_tile,
                            ],
                        )
                    return dst

                from concourse.kernels.tile_matmul import ShapeInfo

                return producer, ShapeInfo(pdims=[(128, k_dim // 128)], fdims=[OUT_F])

            for _ in range(repeats):
                if w_cache:
                    # per-repeat cache + memo: each bench pass re-streams W
                    # once, so repeat-marginal timing matches a real pass
                    w_sbuf = kxn_pool.tile(
                        [128, IN_F // 128, OUT_F],
                        mybir.dt.float8e4,
                        name="w_sbuf",
                        tag="w_sbuf",
                    )
                    w_loaded = set()
                kxm_prods, kxm_shapes = [], []
                kxn_prods, kxn_shapes = [], []
                for (ap, ksl), pool in zip(x_parts, kxm_pools):
                    p, s = dma_from_dram_kxm(pool, ap)
                    kxm_prods.append(p)
                    kxm_shapes.append(s)
                    if w_cache:
                        p, s = make_cached_kxn(
                            w_sbuf, w_loaded, ksl.start // 128,
                            ksl.stop - ksl.start,
                        )
                    else:
                        p, s = dma_from_dram_kxn(kxn_pool, w_t[ksl])
                    kxn_prods.append(p)
                    kxn_shapes.append(s)
                kxm_producer, kxm_shape = batched_producer_kxm(
                    kxm_prods, kxm_shapes, batch_dim="k"
                )
                kxn_producer, kxn_shape = batched_producer_kxn(
                    kxn_prods, kxn_shapes, batch_dim="k"
                )
                extra = {}
                if split_out:
                    # Evict+store per PSUM subtile: each subtile's DRAM DMA
                    # starts right after its own PSUM->SBUF copy instead of
                    # after the whole block's 4 copies — pipelines the
                    # last-block tail and spreads output DMAs.
                    y3 = y.rearrange("(po pi) f -> pi po f", pi=128)

                    def reducer(nc_, psum, sbuf, md):
                        # alternate engines so the block's 4 evictions run
                        # pairwise-parallel (GpSimd can't read PSUM)
                        if md.m_subtile_idx % 2 == 0:
                            nc_.vector.tensor_copy(out=sbuf, in_=psum)
                        else:
                            nc_.scalar.copy(out=sbuf, in_=psum)
                        n_sz = min(
                            md.n_subtile,
                            md.n_slice_size - md.n_subtile_idx * md.n_subtile,
                        )
                        nc_.sync.dma_start(
                            y3[
                                :,
                                md.m_tile_idx * md.m_subtiles + md.m_subtile_idx,
                                bass.ds(
                                    md.n_tile_idx * md.n_tile
                                    + md.n_subtile_idx * md.n_subtile,
                                    n_sz,
                                ),
                            ],
                            sbuf[:, 0, :n_sz],
                        )

                    extra["mxn_subtile_reducer"] = reducer
                    mxn_consumer = lambda nc_, tile_, md: None
                else:
                    mxn_consumer = dma_to_dram_mxn(y)
                composable_matmul_tile_kernel(
                    tc=tc,
                    kxm_shape=kxm_shape,
                    kxn_shape=kxn_shape,
                    output_type=y.dtype,
                    kxm_producer=kxm_producer,
                    kxn_producer=kxn_producer,
                    mxn_consumer=mxn_consumer,
                    MATMUL_FREE_DIM=free_dim,
                    MAX_TILE_SIZE=max_tile,
                    MAX_K_TILE_SIZE=max_k_tile,
                    temps_n_bufs=temps_bufs,
                    psum_n_bufs=psum_bufs,
                    **extra,
                )
    nc.compile()
    return nc


def _get_nc():
    if "nc" not in _C:
        _C["nc"] = _build_nc()
    return _C["nc"]


def _in_names(nc):
    import concourse.mybir as mybir

    partition_name = nc.partition_id_tensor.name if nc.partition_id_tensor else None
    names = []
    for alloc in nc.m.functions[0].allocations:
        if not isinstance(alloc, mybir.MemoryLocationSet):
            continue
        name = alloc.memorylocations[0].name
        if alloc.kind == "ExternalInput" and name != partition_name:
            names.append(name)
    return names


def _get_runner():
    """Compile the 8-core jitted executable once; returns (fn, zeros_fn)."""
    if "runner" in _C:
        return _C["runner"]
    import jax
    import jax.numpy as jnp
    from jax.sharding import Mesh, NamedSharding, PartitionSpec

    import inspect

    try:
        from jax.experimental.shard_map import shard_map
    except ImportError:
        from jax import shard_map
    _rep_kw = (
        {"check_rep": False}
        if "check_rep" in inspect.signature(shard_map).parameters
        else {"check_vma": False}
    )
    import concourse.mybir as mybir
    from concourse import bass2jax
    from concourse.bass2jax import _bass_exec_p, install_neuronx_cc_hook

    nc = _get_nc()
    install_neuronx_cc_hook()

    partition_name = nc.partition_id_tensor.name if nc.partition_id_tensor else None
    in_names, out_names, out_avals = [], [], []
    for alloc in nc.m.functions[0].allocations:
        if not isinstance(alloc, mybir.MemoryLocationSet):
            continue
        name = alloc.memorylocations[0].name
        if alloc.kind == "ExternalInput":
            if name != partition_name:
                in_names.append(name)
        elif alloc.kind == "ExternalOutput":
            out_names.append(name)
            out_avals.append(
                jax.core.ShapedArray(
                    tuple(alloc.tensor_shape), mybir.dt.np(alloc.dtype)
                )
            )
    expect = (["x8_t"] if K8 else []) + (["xb_t"] if KB else []) + ["w_t"]
    assert in_names == expect and out_names == ["y"], (in_names, out_names)
    all_in_names = list(in_names) + list(out_names)
    if partition_name is not None:
        all_in_names.append(partition_name)

    def _body(*args):
        operands = list(args)
        if partition_name is not None:
            operands.append(bass2jax.partition_id_tensor())
        outs = _bass_exec_p.bind(
            *operands,
            out_avals=tuple(out_avals),
            in_names=tuple(all_in_names),
            out_names=tuple(out_names),
            lowering_input_output_aliases=(),
            sim_require_finite=True,
            sim_require_nnan=True,
            nc=nc,
        )
        return tuple(outs)

    devices = jax.devices()[:N_CORES]
    mesh = Mesh(np.asarray(devices), ("core",))
    sharding = NamedSharding(mesh, PartitionSpec("core"))
    n_args = len(in_names) + 1  # inputs + y backing
    in_specs = (PartitionSpec("core"),) * n_args
    out_specs = (PartitionSpec("core"),)
    fn = jax.jit(
        shard_map(_body, mesh=mesh, in_specs=in_specs, out_specs=out_specs,
                  **_rep_kw),
        donate_argnums=(n_args - 1,),
        keep_unused=True,
    )
    out_np_dt = out_avals[0].dtype
    zeros_fn = jax.jit(
        lambda: jnp.zeros((N_TOKENS, OUT_F), out_np_dt),
        out_shardings=sharding,
    )
    _C["runner"] = (fn, zeros_fn, sharding, jax)
    return _C["runner"]


def _shard_cols(xt):
    """[K, 8192] -> [8*K, 1024] global stacked layout for shard_map."""
    return np.concatenate(
        [xt[:, c * TOK_SHARD : (c + 1) * TOK_SHARD] for c in range(N_CORES)],
        axis=0,
    )


def _host_prep(x, weight):
    """sign/transpose/cast/shard on the host (cheap vs the matmul).

    Returns the global (8-core stacked) arrays in kernel input order:
    x8_t (fp8 K-slice), xb_t (bf16 K-slice), w_t.
    """
    xt = np.ascontiguousarray(np.asarray(x).T)
    parts = []
    if K8:
        parts.append(
            _shard_cols(xt[:K8].astype(ml_dtypes.float8_e4m3))
        )
    if KB:
        parts.append(
            _shard_cols(xt[K8:].astype(ml_dtypes.bfloat16))
        )
    wt = np.ascontiguousarray(np.sign(np.asarray(weight)).T).astype(
        ml_dtypes.float8_e4m3
    )
    parts.append(np.concatenate([wt] * N_CORES, axis=0))
    return parts


def _run_spmd_fallback(x, weight):
    """Conservative path through bass_utils.run_bass_kernel_spmd (same
    underlying bass2jax/PJRT execution; pays extra host->device bytes for the
    zero-filled output backing buffers)."""
    from concourse.bass_utils import run_bass_kernel_spmd

    nc = _get_nc()
    xt = np.ascontiguousarray(np.asarray(x).T)
    x8 = xt[:K8].astype(ml_dtypes.float8_e4m3) if K8 else None
    xb = xt[K8:].astype(ml_dtypes.bfloat16) if KB else None
    wt = np.ascontiguousarray(np.sign(np.asarray(weight)).T).astype(
        ml_dtypes.float8_e4m3
    )
    in_maps = []
    for c in range(N_CORES):
        sl = slice(c * TOK_SHARD, (c + 1) * TOK_SHARD)
        m = {}
        if x8 is not None:
            m["x8_t"] = np.ascontiguousarray(x8[:, sl])
        if xb is not None:
            m["xb_t"] = np.ascontiguousarray(xb[:, sl])
        m["w_t"] = wt
        in_maps.append(m)
    res = run_bass_kernel_spmd(nc, in_maps, core_ids=list(range(N_CORES)))
    return np.concatenate([r["y"] for r in res.results], axis=0)


def kernel(x, weight, bias):
    try:
        fn, zeros_fn, sharding, jax = _get_runner()
        parts = _host_prep(x, weight)
        args = [jax.device_put(p, sharding) for p in parts]
        args.append(zeros_fn())
        (yd,) = fn(*args)
        # global [8192, 4096], token order preserved
        y = np.asarray(yd)
    except Exception:
        y = _run_spmd_fallback(x, weight)
    # upcast + bias on host
    y = y.astype(np.float32)
    y += np.asarray(bias, dtype=np.float32)[None, :]
    return y


# revision 7
# speedup vs baseline: 1.5624x; 1.0362x over previous
# BinaryLinear on 8 Trainium2 NeuronCores.
#
# y = x @ sign(W)^T + bias for x [8192, 4096] f32, W [4096, 4096] f32.
#
# Sharding: data-parallel over the 8192 tokens (1024 per core). Each core
# runs one [K=4096, M=1024] x [K=4096, N=4096] matmul entirely in fp8e4m3
# DoubleRow perf mode (PE virtualizes to 128x256; measured ~3.5x the bf16
# row rate on this hardware), with a two-term x quantization:
#   - batch A: hi = e4m3(x), full K=4096, against full W.
#   - batch B: lo = e4m3(x - hi) on the first KLO = N_LO*256 columns,
#     against the same W rows (error-feedback correction).
# +-1 weights are exact in fp8; accumulation is fp32 in PSUM across both
# batches. Corrected columns carry only the second-order residual, so
# rel err ~= sqrt(1 - KLO/4096)*0.0266; N_LO=9 measures 0.0175 on the
# reference data vs the 2e-2 gate. W is DMA'd once into a full-size SBUF
# cache (16.8MB) shared by both batches and both m-tile passes.
#
# Execution goes through bass2jax/PJRT (axon): one jitted shard_map over the
# 8-core mesh. The donated output backing buffer is created on-device so no
# zero-filled bytes cross the host->device link.

import numpy as np
import ml_dtypes

N_TOKENS = 8192
IN_F = 4096
OUT_F = 4096
N_CORES = 8
TOK_SHARD = N_TOKENS // N_CORES

N_LO = 9  # number of 256-wide K chunks getting the lo correction term
KLO = N_LO * 256

_C = {}


OUT_DT = "float16"  # device-side output dtype (upcast to f32 on host).
# f16 keeps D2H small; rounding f32 PSUM results to f16 adds ~3e-4 relative
# error rms on top of the x-quantization error — negligible.


def _build_nc(
    out_dt=None,
    n_lo=None,
    # 256 keeps first-tile DMA small AND gives K_SUBTILES=2, which is what
    # lets the composable kernel emit DoubleRow ([128,2,*] slices) for the
    # fp8 batches.
    max_k_tile=256,
    max_tile=512,
    free_dim=512,
    repeats=1,
    psum_bufs=1,
    temps_bufs=3,
    n_warm=5,
    split_out=True,
    w_cache=True,
):
    import concourse.mybir as mybir
    import concourse.tile as tile
    from concourse import bacc
    from concourse.kernels.tile_matmul import (
        batched_producer_kxm,
        batched_producer_kxn,
        composable_matmul_tile_kernel,
        dma_from_dram_kxm,
        dma_from_dram_kxn,
        dma_to_dram_mxn,
    )

    out_dt = out_dt or OUT_DT
    n_lo = N_LO if n_lo is None else n_lo
    klo = n_lo * 256
    nc = bacc.Bacc("TRN2", target_bir_lowering=False, debug=False)
    x_parts = []  # (ap, k-slice into w)
    x8_t = nc.dram_tensor(
        "x8_t", [IN_F, TOK_SHARD], mybir.dt.float8e4, kind="ExternalInput"
    ).ap()
    x_parts.append((x8_t, slice(0, IN_F)))
    if klo:
        xl_t = nc.dram_tensor(
            "xl_t", [klo, TOK_SHARD], mybir.dt.float8e4, kind="ExternalInput"
        ).ap()
        x_parts.append((xl_t, slice(0, klo)))
    w_t = nc.dram_tensor(
        "w_t", [IN_F, OUT_F], mybir.dt.float8e4, kind="ExternalInput"
    ).ap()
    y = nc.dram_tensor(
        "y", [TOK_SHARD, OUT_F], getattr(mybir.dt, out_dt), kind="ExternalOutput"
    ).ap()
    with tile.TileContext(nc) as tc:
        import contextlib

        with contextlib.ExitStack() as es:
            if n_warm:
                # PE warm-up: dependency-free dummy matmuls on memset tiles
                # run while the first input DMAs are in flight, so the real
                # matmul stream starts past the HAM/pstate ramp (the PE runs
                # at half clock until ~3.4us of sustained activity).
                warm = es.enter_context(tc.tile_pool(name="warm", bufs=1))
                warm_ps = es.enter_context(
                    tc.tile_pool(name="warm_ps", bufs=1, space="PSUM")
                )
                # one bf16 tile doubles as lhsT and rhs: a single fast memset
                # (bf16 SBUF hits the DVE 4x mode) is the only dependency, so
                # the PE starts within ~200ns of kernel entry
                w_t_ = warm.tile([128, 512], mybir.dt.bfloat16)
                nc.vector.memset(w_t_[:], 1.0)
                w_out = warm_ps.tile([128, 512], mybir.dt.float32)
                for _ in range(n_warm):
                    nc.tensor.matmul(
                        w_out[:], w_t_[:, :128], w_t_[:], start=True, stop=True
                    )
            # per-batch kxm pools sized to their k-tile counts (+1 rotation
            # slack); one shared kxn pool (same tag: same shape+dtype).
            kxm_pools = [
                es.enter_context(
                    tc.tile_pool(
                        name=f"kxm_pool{i}",
                        bufs=(ap.shape[0] // max_k_tile) + 1,
                    )
                )
                for i, (ap, _) in enumerate(x_parts)
            ]
            kxn_pool = es.enter_context(
                tc.tile_pool(
                    name="kxn_pool",
                    bufs=1 if w_cache else (IN_F // max_k_tile) + 1,
                )
            )
            import concourse.bass as bass

            # [p, ko, n] view of w_t: row k = ko*128 + p
            w3 = w_t.rearrange("(ko ki) n -> ki ko n", ki=128)

            def make_cached_kxn(cache, loaded, base_subtile, k_dim):
                """Producer DMA-ing W k-slices into a persistent SBUF cache
                exactly once; repeat calls (second m_outer pass) return the
                cached slice with no DMA. Halves W HBM traffic vs streaming."""

                def producer(nc_, md):
                    ks = base_subtile + md.k_tile_idx * md.k_subtiles
                    dst = cache[
                        :,
                        ks : ks + md.k_subtiles,
                        md.n_tile_idx * md.n_tile : md.n_tile_idx * md.n_tile
                        + md.n_tile,
                    ]
                    key = (ks, md.n_tile_idx)
                    if key not in loaded:
                        loaded.add(key)
                        nc_.sync.dma_start(
                            dst,
                            w3[
                                :,
                                ks : ks + md.k_subtiles,
                                md.n_tile_idx * md.n_tile : md.n_tile_idx
                                * md.n_tile
                                + md.n_tile,
                            ],
                        )
                    return dst

                from concourse.kernels.tile_matmul import ShapeInfo

                return producer, ShapeInfo(pdims=[(128, k_dim // 128)], fdims=[OUT_F])

            for _ in range(repeats):
                if w_cache:
                    # per-repeat cache + memo: each bench pass re-streams W
                    # once, so repeat-marginal timing matches a real pass
                    w_sbuf = kxn_pool.tile(
                        [128, IN_F // 128, OUT_F],
                        mybir.dt.float8e4,
                        name="w_sbuf",
                        tag="w_sbuf",
                    )
                    w_loaded = set()
                kxm_prods, kxm_shapes = [], []
                kxn_prods, kxn_shapes = [], []
                for (ap, ksl), pool in zip(x_parts, kxm_pools):
                    p, s = dma_from_dram_kxm(pool, ap)
                    kxm_prods.append(p)
                    kxm_shapes.append(s)
                    if w_cache:
                        p, s = make_cached_kxn(
                            w_sbuf, w_loaded, ksl.start // 128,
                            ksl.stop - ksl.start,
                        )
                    else:
                        p, s = dma_from_dram_kxn(kxn_pool, w_t[ksl])
                    kxn_prods.append(p)
                    kxn_shapes.append(s)
                kxm_producer, kxm_shape = batched_producer_kxm(
                    kxm_prods, kxm_shapes, batch_dim="k"
                )
                kxn_producer, kxn_shape = batched_producer_kxn(
                    kxn_prods, kxn_shapes, batch_dim="k"
                )
                extra = {}
                if split_out:
                    # Evict+store per PSUM subtile: each subtile's DRAM DMA
                    # starts right after its own PSUM->SBUF copy instead of
                    # after the whole block's 4 copies — pipelines the
                    # last-block tail and spreads output DMAs.
                    y3 = y.rearrange("(po pi) f -> pi po f", pi=128)

                    def reducer(nc_, psum, sbuf, md):
                        # alternate engines so the block's 4 evictions run
                        # pairwise-parallel (GpSimd can't read PSUM)
                        if md.m_subtile_idx % 2 == 0:
                            nc_.vector.tensor_copy(out=sbuf, in_=psum)
                        else:
                            nc_.scalar.copy(out=sbuf, in_=psum)
                        n_sz = min(
                            md.n_subtile,
                            md.n_slice_size - md.n_subtile_idx * md.n_subtile,
                        )
                        nc_.sync.dma_start(
                            y3[
                                :,
                                md.m_tile_idx * md.m_subtiles + md.m_subtile_idx,
                                bass.ds(
                                    md.n_tile_idx * md.n_tile
                                    + md.n_subtile_idx * md.n_subtile,
                                    n_sz,
                                ),
                            ],
                            sbuf[:, 0, :n_sz],
                        )

                    extra["mxn_subtile_reducer"] = reducer
                    mxn_consumer = lambda nc_, tile_, md: None
                else:
                    mxn_consumer = dma_to_dram_mxn(y)
                composable_matmul_tile_kernel(
                    tc=tc,
                    kxm_shape=kxm_shape,
                    kxn_shape=kxn_shape,
                    output_type=y.dtype,
                    kxm_producer=kxm_producer,
                    kxn_producer=kxn_producer,
                    mxn_consumer=mxn_consumer,
                    MATMUL_FREE_DIM=free_dim,
                    MAX_TILE_SIZE=max_tile,
                    MAX_K_TILE_SIZE=max_k_tile,
                    temps_n_bufs=temps_bufs,
                    psum_n_bufs=psum_bufs,
                    **extra,
                )
    nc.compile()
    return nc


def _get_nc():
    if "nc" not in _C:
        _C["nc"] = _build_nc()
    return _C["nc"]


def _in_names(nc):
    import concourse.mybir as mybir

    partition_name = nc.partition_id_tensor.name if nc.partition_id_tensor else None
    names = []
    for alloc in nc.m.functions[0].allocations:
        if not isinstance(alloc, mybir.MemoryLocationSet):
            continue
        name = alloc.memorylocations[0].name
        if alloc.kind == "ExternalInput" and name != partition_name:
            names.append(name)
    return names


def _get_runner():
    """Compile the 8-core jitted executable once; returns (fn, zeros_fn)."""
    if "runner" in _C:
        return _C["runner"]
    import jax
    import jax.numpy as jnp
    from jax.sharding import Mesh, NamedSharding, PartitionSpec

    import inspect

    try:
        from jax.experimental.shard_map import shard_map
    except ImportError:
        from jax import shard_map
    _rep_kw = (
        {"check_rep": False}
        if "check_rep" in inspect.signature(shard_map).parameters
        else {"check_vma": False}
    )
    import concourse.mybir as mybir
    from concourse import bass2jax
    from concourse.bass2jax import _bass_exec_p, install_neuronx_cc_hook

    nc = _get_nc()
    install_neuronx_cc_hook()

    partition_name = nc.partition_id_tensor.name if nc.partition_id_tensor else None
    in_names, out_names, out_avals = [], [], []
    for alloc in nc.m.functions[0].allocations:
        if not isinstance(alloc, mybir.MemoryLocationSet):
            continue
        name = alloc.memorylocations[0].name
        if alloc.kind == "ExternalInput":
            if name != partition_name:
                in_names.append(name)
        elif alloc.kind == "ExternalOutput":
            out_names.append(name)
            out_avals.append(
                jax.core.ShapedArray(
                    tuple(alloc.tensor_shape), mybir.dt.np(alloc.dtype)
                )
            )
    expect = ["x8_t"] + (["xl_t"] if KLO else []) + ["w_t"]
    assert in_names == expect and out_names == ["y"], (in_names, out_names)
    all_in_names = list(in_names) + list(out_names)
    if partition_name is not None:
        all_in_names.append(partition_name)

    def _body(*args):
        operands = list(args)
        if partition_name is not None:
            operands.append(bass2jax.partition_id_tensor())
        outs = _bass_exec_p.bind(
            *operands,
            out_avals=tuple(out_avals),
            in_names=tuple(all_in_names),
            out_names=tuple(out_names),
            lowering_input_output_aliases=(),
            sim_require_finite=True,
            sim_require_nnan=True,
            nc=nc,
        )
        return tuple(outs)

    devices = jax.devices()[:N_CORES]
    mesh = Mesh(np.asarray(devices), ("core",))
    sharding = NamedSharding(mesh, PartitionSpec("core"))
    n_args = len(in_names) + 1  # inputs + y backing
    in_specs = (PartitionSpec("core"),) * n_args
    out_specs = (PartitionSpec("core"),)
    fn = jax.jit(
        shard_map(_body, mesh=mesh, in_specs=in_specs, out_specs=out_specs,
                  **_rep_kw),
        donate_argnums=(n_args - 1,),
        keep_unused=True,
    )
    out_np_dt = out_avals[0].dtype
    zeros_fn = jax.jit(
        lambda: jnp.zeros((N_TOKENS, OUT_F), out_np_dt),
        out_shardings=sharding,
    )
    _C["runner"] = (fn, zeros_fn, sharding, jax)
    return _C["runner"]


def _shard_cols(xt):
    """[K, 8192] -> [8*K, 1024] global stacked layout for shard_map."""
    return np.concatenate(
        [xt[:, c * TOK_SHARD : (c + 1) * TOK_SHARD] for c in range(N_CORES)],
        axis=0,
    )


def _host_prep(x, weight):
    """sign/transpose/cast/shard on the host (cheap vs the matmul).

    Returns the global (8-core stacked) arrays in kernel input order:
    x8_t (fp8 K-slice), xb_t (bf16 K-slice), w_t.
    """
    xt = np.ascontiguousarray(np.asarray(x).T)
    hi = xt.astype(ml_dtypes.float8_e4m3)
    parts = [_shard_cols(hi)]
    if KLO:
        lo = (xt[:KLO] - hi[:KLO].astype(np.float32)).astype(
            ml_dtypes.float8_e4m3
        )
        parts.append(_shard_cols(lo))
    wt = np.ascontiguousarray(np.sign(np.asarray(weight)).T).astype(
        ml_dtypes.float8_e4m3
    )
    parts.append(np.concatenate([wt] * N_CORES, axis=0))
    return parts


def _run_spmd_fallback(x, weight):
    """Conservative path through bass_utils.run_bass_kernel_spmd (same
    underlying bass2jax/PJRT execution; pays extra host->device bytes for the
    zero-filled output backing buffers)."""
    from concourse.bass_utils import run_bass_kernel_spmd

    nc = _get_nc()
    xt = np.ascontiguousarray(np.asarray(x).T)
    hi = xt.astype(ml_dtypes.float8_e4m3)
    lo = (
        (xt[:KLO] - hi[:KLO].astype(np.float32)).astype(ml_dtypes.float8_e4m3)
        if KLO
        else None
    )
    wt = np.ascontiguousarray(np.sign(np.asarray(weight)).T).astype(
        ml_dtypes.float8_e4m3
    )
    in_maps = []
    for c in range(N_CORES):
        sl = slice(c * TOK_SHARD, (c + 1) * TOK_SHARD)
        m = {"x8_t": np.ascontiguousarray(hi[:, sl])}
        if lo is not None:
            m["xl_t"] = np.ascontiguousarray(lo[:, sl])
        m["w_t"] = wt
        in_maps.append(m)
    res = run_bass_kernel_spmd(nc, in_maps, core_ids=list(range(N_CORES)))
    return np.concatenate([r["y"] for r in res.results], axis=0)


def kernel(x, weight, bias):
    try:
        fn, zeros_fn, sharding, jax = _get_runner()
        parts = _host_prep(x, weight)
        args = [jax.device_put(p, sharding) for p in parts]
        args.append(zeros_fn())
        (yd,) = fn(*args)
        # global [8192, 4096], token order preserved
        y = np.asarray(yd)
    except Exception:
        y = _run_spmd_fallback(x, weight)
    # upcast + bias on host
    y = y.astype(np.float32)
    y += np.asarray(bias, dtype=np.float32)[None, :]
    return y
